# revision 2
# baseline (speedup 1.0000x reference)
"""Distributed 3-layer GraphSAGE (mean aggregator) on 8 TRN2 NeuronCores.

Strategy (graph/data parallel, per spec sharding hint):
  - Host: relabel nodes into 8 cores x 40 windows of 125 nodes with balanced
    in-degree; sort edges by (core, window, src-class); pad each (window,
    class) run to uniform tile counts -> fully static SPMD program.
  - Device, per layer: dma_gather edge source rows from a replicated
    node-major feature table in HBM; segment-sum via one-hot selection
    matrices (built on DVE, inv-degree folded in) multiplied on the
    TensorEngine into PSUM per window; transform = W matmuls with
    feature-major activations; AllGather rebuilds the replicated table
    between layers.
  - Layer 3 uses transform-before-aggregate (m3 = h2 @ W3_bot, 47->64 pad)
    so the edge gather moves 64-wide rows instead of 256.
"""
import numpy as np

import concourse.bacc as bacc
import concourse.mybir as mybir
import concourse.tile as tile
from concourse import bass
from concourse.bass_utils import run_bass_kernel_spmd
from concourse.library_config import mlp
from concourse.masks import make_identity

# ---- problem constants (hardcoded per contest rules) ----
N = 40000
E = 640000
DIN, HID, DOUT = 128, 256, 47
M3P = 64          # padded width of layer-3 edge features
NCORES = 8
WN = 125          # nodes per window (<= 128 PSUM partitions)
NW = 40           # windows per core
NPC = WN * NW     # 5000 nodes per core
SPLIT = 20000     # edge class split on src new-id (core-boundary aligned)
PAD_LOC = 126     # dead psum row for padding edges
CT = 8            # gather chunk size (tiles of 128 edges); 1024 idx/call
                  # is the SWDGE descriptor-ring capacity limit per dma_gather

F32 = mybir.dt.float32
BF16 = mybir.dt.bfloat16
I16 = mybir.dt.int16
AF = mybir.ActivationFunctionType
ALU = mybir.AluOpType

LAST_EXEC_NS = None
LAST_RESULT = None


# ======================= host-side planning =======================

def _plan(src, dst):
    import heapq
    src = np.asarray(src, dtype=np.int64)
    dst = np.asarray(dst, dtype=np.int64)
    deg = np.bincount(dst, minlength=N).astype(np.int64)

    nbins = NCORES * NW
    order = np.argsort(-deg, kind="stable")
    heap = [(0, b) for b in range(nbins)]
    heapq.heapify(heap)
    counts = np.zeros(nbins, dtype=np.int64)
    bin_of = np.empty(N, dtype=np.int64)
    spill = []
    for n in order:
        while True:
            load, b = heapq.heappop(heap)
            if counts[b] < WN:
                break
            spill.append((load, b))
        bin_of[n] = b
        counts[b] += 1
        if counts[b] < WN:
            heapq.heappush(heap, (load + int(deg[n]), b))
        for item in spill:
            heapq.heappush(heap, item)
        spill.clear()

    slot_in_bin = np.zeros(nbins, dtype=np.int64)
    perm = np.empty(N, dtype=np.int64)  # old -> new
    for n in range(N):
        b = bin_of[n]
        perm[n] = (b // NW) * NPC + (b % NW) * WN + slot_in_bin[b]
        slot_in_bin[b] += 1
    inv_perm = np.empty(N, dtype=np.int64)
    inv_perm[perm] = np.arange(N)

    srcN = perm[src]
    dstN = perm[dst]
    invdeg = np.zeros(N, dtype=np.float32)
    nz = deg > 0
    invdeg[nz] = (1.0 / deg[nz]).astype(np.float32)
    invdegN = invdeg[inv_perm]

    core_e = dstN // NPC
    win_e = (dstN % NPC) // WN
    loc_e = dstN % WN
    cls_e = (srcN >= SPLIT).astype(np.int64)
    key = (core_e * NW + win_e) * 2 + cls_e
    order_e = np.argsort(key, kind="stable")
    key_s = key[order_e]
    srcN_s = srcN[order_e]
    loc_s = loc_e[order_e]
    cnt = np.bincount(key_s, minlength=nbins * 2)
    starts = np.zeros(nbins * 2 + 1, dtype=np.int64)
    np.cumsum(cnt, out=starts[1:])

    T_A = int(np.ceil(cnt[0::2].max() / 128))
    T_B = int(np.ceil(cnt[1::2].max() / 128))
    LA, LB = NW * T_A * 128, NW * T_B * 128
    L = LA + LB
    NT = L // 128

    idx16 = np.zeros((NCORES, L), dtype=np.int16)
    dstloc = np.full((NCORES, L), PAD_LOC, dtype=np.float32)
    invdst = np.zeros((NCORES, L), dtype=np.float32)
    for c in range(NCORES):
        for w in range(NW):
            for s, (T, base_off) in enumerate(((T_A, 0), (T_B, LA))):
                k = (c * NW + w) * 2 + s
                e0, e1 = starts[k], starts[k + 1]
                n = e1 - e0
                off = base_off + w * T * 128
                sv = srcN_s[e0:e1]
                idx16[c, off:off + n] = (sv - (SPLIT if s else 0)).astype(np.int16)
                dstloc[c, off:off + n] = loc_s[e0:e1].astype(np.float32)
                dst_new = c * NPC + w * WN + loc_s[e0:e1]
                invdst[c, off:off + n] = invdegN[dst_new]

    idx_pack = np.empty((NCORES, 128, L // 16), dtype=np.int16)
    dstloc_pack = np.empty((NCORES, 128, NT), dtype=np.float32)
    invdst_pack = np.empty((NCORES, 128, NT), dtype=np.float32)
    for c in range(NCORES):
        blk = idx16[c].reshape(L // 16, 16).T
        idx_pack[c] = np.tile(blk, (8, 1))
        dstloc_pack[c] = dstloc[c].reshape(NT, 128).T
        invdst_pack[c] = invdst[c].reshape(NT, 128).T

    return dict(
        perm=perm, inv_perm=inv_perm, T_A=T_A, T_B=T_B,
        idx_pack=idx_pack, dstloc_pack=dstloc_pack, invdst_pack=invdst_pack,
    )


def _rearrange_w(W, kchunks):
    """[K, M] -> [128, kchunks*M] with k-chunk blocks along free dim."""
    K, M = W.shape
    assert K == kchunks * 128
    return np.ascontiguousarray(
        W.reshape(kchunks, 128, M).transpose(1, 0, 2).reshape(128, kchunks * M)
    ).astype(np.float32)


# ======================= device program =======================

def _build(T_A, T_B):
    import os
    MAXW = int(os.environ.get("KERNEL_MAXW", NW))
    NLAYERS = int(os.environ.get("KERNEL_NLAYERS", 3))
    nc = bacc.Bacc("TRN2", num_devices=NCORES, num_swdge_queues=2)
    NT_A, NT_B = NW * T_A, NW * T_B
    NT = NT_A + NT_B
    L = NT * 128

    # ---- kernel I/O ----
    x_nm = nc.dram_tensor("x_nm", [N, DIN], F32, kind="ExternalInput")
    xT_own = nc.dram_tensor("xT_own", [128, NPC], F32, kind="ExternalInput")
    idx_d = nc.dram_tensor("idx", [128, L // 16], I16, kind="ExternalInput")
    dstloc_d = nc.dram_tensor("dstloc", [128, NT], F32, kind="ExternalInput")
    invdst_d = nc.dram_tensor("invdst", [128, NT], F32, kind="ExternalInput")
    iota_d = nc.dram_tensor("iota", [128, 128], F32, kind="ExternalInput")
    w1_d = nc.dram_tensor("w1", [128, 2 * HID], F32, kind="ExternalInput")
    w2_d = nc.dram_tensor("w2", [128, 4 * HID], F32, kind="ExternalInput")
    w3t_d = nc.dram_tensor("w3t", [128, 2 * M3P], F32, kind="ExternalInput")
    w3b_d = nc.dram_tensor("w3b", [128, 2 * M3P], F32, kind="ExternalInput")
    b12_d = nc.dram_tensor("b12", [128, 4], F32, kind="ExternalInput")
    b3b_d = nc.dram_tensor("b3b", [128, M3P], F32, kind="ExternalInput")
    out_d = nc.dram_tensor("out", [NPC, DOUT], F32, kind="ExternalOutput")

    with tile.TileContext(nc) as tc:
        with (
            tc.tile_pool(name="persist", bufs=1) as PP,
            tc.tile_pool(name="dram", bufs=1, space="DRAM") as DP,
            tc.tile_pool(name="psA", bufs=2, space="PSUM") as PSA,
            tc.tile_pool(name="psT", bufs=2, space="PSUM") as PST,
            tc.tile_pool(name="ebufA", bufs=3) as PEA,
            tc.tile_pool(name="ebufB", bufs=3) as PEB,
            tc.tile_pool(name="sp", bufs=4) as PSP,
            tc.tile_pool(name="tmp", bufs=2) as PT,
        ):
            nc.gpsimd.load_library(mlp)

            # persistent SBUF
            idx_sb = PP.tile([128, L // 16], I16)
            dstloc_sb = PP.tile([128, NT], F32)
            invdst_sb = PP.tile([128, NT], F32)
            iota_sb = PP.tile([128, 128], F32)
            w1_sb = PP.tile([128, 2 * HID], F32)
            w2_sb = PP.tile([128, 4 * HID], F32)
            w3t_sb = PP.tile([128, 2 * M3P], F32)
            w3b_sb = PP.tile([128, 2 * M3P], F32)
            b12_sb = PP.tile([128, 4], F32)
            b3b_sb = PP.tile([128, M3P], F32)
            ident = PP.tile([128, 128], F32)
            h1T = [PP.tile([128, NPC], F32, name=f"h1T{c}", tag=f"h1T{c}")
                   for c in range(2)]
            h2T = [PP.tile([128, NPC], F32, name=f"h2T{c}", tag=f"h2T{c}")
                   for c in range(2)]

            for sb, dr in ((idx_sb, idx_d), (dstloc_sb, dstloc_d),
                           (invdst_sb, invdst_d), (iota_sb, iota_d),
                           (w1_sb, w1_d), (w2_sb, w2_d), (w3t_sb, w3t_d),
                           (w3b_sb, w3b_d), (b12_sb, b12_d), (b3b_sb, b3b_d)):
                nc.sync.dma_start(sb[:], dr[:])
            make_identity(nc, ident[:])

            # DRAM intermediates
            h1_own = DP.tile([NPC, HID], BF16)
            h1_full = DP.tile([N, HID], BF16)
            m3_own = DP.tile([NPC, M3P], F32)
            m3_full = DP.tile([N, M3P], F32)

            # ---------- generic aggregation pass ----------
            def agg_layer(tableA, tableB, d, edt, epilogue, stop_in_agg=True):
                """For each window: psum[seg, d] = sum_e S'[e,seg]^T E[e, d]
                with inv-degree folded into S'. Calls epilogue(w, psum)."""
                issued = [0, 0]   # chunks issued per class
                bufs = [{}, {}]   # chunk idx -> (tile, tiles_in_chunk)
                streams = (
                    (0, T_A, 0, NT_A, tableA, PEA),
                    (1, T_B, NT_A, NT_B, tableB, PEB),
                )

                def ensure_chunk(s, tix):
                    _, T, tile_off, nt, table, pool = streams[s]
                    c = tix // CT
                    while issued[s] <= c:
                        cc = issued[s]
                        t0 = cc * CT
                        ctn = min(CT, nt - t0)
                        ebuf = pool.tile([128, CT * d], edt, tag=f"eb{s}")
                        col0 = (tile_off + t0) * 8  # 128 idx / 16 per col
                        nidx = ctn * 128
                        nc.gpsimd.dma_gather(
                            ebuf[:, :ctn * d].rearrange("p (t e) -> p t e", e=d),
                            table,
                            idx_sb[:, col0:col0 + nidx // 16],
                            nidx, nidx, d,
                            queue_num=s,
                        )
                        bufs[s][cc] = ebuf
                        issued[s] += 1
                    return bufs[s][c]

                for w in range(min(NW, MAXW)):
                    psum = PSA.tile([128, d], F32, tag="agg")
                    n_ent = T_A + T_B
                    i = 0
                    for s, T, tile_off, nt, table, pool in streams:
                        for j in range(T):
                            tix = w * T + j
                            ebuf = ensure_chunk(s, tix)
                            slot = tix % CT
                            col = tile_off + tix
                            sp = PSP.tile([128, 128], edt, tag="sp")
                            nc.vector.tensor_scalar(
                                sp[:], iota_sb[:],
                                dstloc_sb[:, col:col + 1],
                                invdst_sb[:, col:col + 1],
                                ALU.is_equal, ALU.mult,
                            )
                            nc.tensor.matmul(
                                psum[:], lhsT=sp[:],
                                rhs=ebuf[:, slot * d:(slot + 1) * d],
                                start=(i == 0),
                                stop=(stop_in_agg and i == n_ent - 1),
                            )
                            i += 1
                    epilogue(w, psum)

            # ---------- layer 1 ----------
            def epi1(w, psum):
                ws = slice(w * WN, (w + 1) * WN)
                mean_w = PT.tile([128, DIN], F32, tag="mean1")
                nc.vector.tensor_copy(mean_w[:], psum[:])
                pt = PST.tile([128, 128], F32, tag="tr")
                nc.tensor.transpose(pt[:], mean_w[:], ident[:])
                meanT = PT.tile([128, 128], F32, tag="meanT1")
                nc.vector.tensor_copy(meanT[:], pt[:])
                xT_w = PT.tile([128, WN], F32, tag="xTw")
                nc.sync.dma_start(xT_w[:], xT_own[:, ws])
                h1nm = PT.tile([128, HID], BF16, tag="h1nm")
                for dc in range(2):
                    ptr = PST.tile([128, WN], F32, tag="tr2")
                    nc.tensor.matmul(ptr[:], lhsT=w1_sb[:, dc * 128:dc * 128 + 128],
                                     rhs=xT_w[:], start=True, stop=False)
                    nc.tensor.matmul(ptr[:], lhsT=w1_sb[:, HID + dc * 128:HID + dc * 128 + 128],
                                     rhs=meanT[:, :WN], start=False, stop=True)
                    nc.scalar.activation(h1T[dc][:, ws], ptr[:], AF.Relu,
                                         bias=b12_sb[:, dc:dc + 1])
                    pt2 = PST.tile([128, 128], F32, tag="tr")
                    nc.tensor.transpose(pt2[:WN, :], h1T[dc][:, ws], ident[:])
                    nc.vector.tensor_copy(h1nm[:WN, dc * 128:dc * 128 + 128], pt2[:WN, :])
                nc.sync.dma_start(h1_own[w * WN:(w + 1) * WN, :], h1nm[:WN, :])

            agg_layer(x_nm[:], x_nm[SPLIT:, :], DIN, F32, epi1)
            if NLAYERS >= 2:
                nc.gpsimd.collective_compute(
                    "AllGather", ALU.bypass,
                    replica_groups=[list(range(NCORES))],
                    ins=[h1_own.opt()], outs=[h1_full.opt()],
                )

            # ---------- layer 2 (+ m3 transform) ----------
            def epi2(w, psum):
                ws = slice(w * WN, (w + 1) * WN)
                mean_w = PT.tile([128, HID], F32, tag="mean2")
                nc.vector.tensor_copy(mean_w[:], psum[:])
                meanT = PT.tile([128, 2 * 128], F32, tag="meanT2")
                for dc in range(2):
                    pt = PST.tile([128, 128], F32, tag="tr")
                    nc.tensor.transpose(pt[:], mean_w[:, dc * 128:(dc + 1) * 128], ident[:])
                    nc.vector.tensor_copy(meanT[:, dc * 128:(dc + 1) * 128], pt[:])
                for dc in range(2):
                    ptr = PST.tile([128, WN], F32, tag="tr2")
                    for k in range(2):   # h1T chunks
                        nc.tensor.matmul(
                            ptr[:], lhsT=w2_sb[:, k * HID + dc * 128:k * HID + dc * 128 + 128],
                            rhs=h1T[k][:, ws], start=(k == 0), stop=False)
                    for k in range(2):   # meanT chunks
                        nc.tensor.matmul(
                            ptr[:], lhsT=w2_sb[:, (2 + k) * HID + dc * 128:(2 + k) * HID + dc * 128 + 128],
                            rhs=meanT[:, k * 128:k * 128 + WN], start=False, stop=(k == 1))
                    nc.scalar.activation(h2T[dc][:, ws], ptr[:], AF.Relu,
                                         bias=b12_sb[:, 2 + dc:3 + dc])
                # m3 = h2 @ W3_bot  (feature-major then node-major)
                pm = PST.tile([128, WN], F32, tag="tr2")
                for k in range(2):
                    nc.tensor.matmul(pm[:M3P, :], lhsT=w3b_sb[:, k * M3P:(k + 1) * M3P],
                                     rhs=h2T[k][:, ws], start=(k == 0), stop=(k == 1))
                m3T_w = PT.tile([128, WN], F32, tag="m3T")
                nc.vector.tensor_copy(m3T_w[:M3P, :], pm[:M3P, :])
                pt3 = PST.tile([128, 128], F32, tag="tr")
                nc.tensor.transpose(pt3[:WN, :M3P], m3T_w[:M3P, :], ident[:M3P, :M3P])
                m3nm = PT.tile([128, M3P], F32, tag="m3nm")
                nc.vector.tensor_copy(m3nm[:WN, :], pt3[:WN, :M3P])
                nc.sync.dma_start(m3_own[w * WN:(w + 1) * WN, :], m3nm[:WN, :])

            if NLAYERS >= 2:
                agg_layer(h1_full[:], h1_full[SPLIT:, :], HID, BF16, epi2)
            if NLAYERS >= 3:
                nc.gpsimd.collective_compute(
                    "AllGather", ALU.bypass,
                    replica_groups=[list(range(NCORES))],
                    ins=[m3_own.opt()], outs=[m3_full.opt()],
                )

            # ---------- layer 3 ----------
            def epi3(w, psum):
                # psum holds mean(m3) [seg, M3P]; accumulate the self term
                # h2 @ W3_top into the same psum, then add bias and store.
                ws = slice(w * WN, (w + 1) * WN)
                for k in range(2):
                    nc.tensor.matmul(psum[:WN, :], lhsT=h2T[k][:, ws],
                                     rhs=w3t_sb[:, k * M3P:(k + 1) * M3P],
                                     start=False, stop=(k == 1))
                out_w = PT.tile([128, DOUT], F32, tag="outw")
                nc.vector.tensor_tensor(out_w[:WN, :], psum[:WN, :DOUT],
                                        b3b_sb[:WN, :DOUT], op=ALU.add)
                nc.sync.dma_start(out_d[w * WN:(w + 1) * WN, :], out_w[:WN, :])

            if NLAYERS >= 3:
                agg_layer(m3_full[:], m3_full[SPLIT:, :], M3P, F32, epi3,
                          stop_in_agg=False)

    nc.compile()
    return nc


# ======================= top-level entry =======================

def _prepare(x, W1, b1, W2, b2, W3, b3, src, dst):
    x = np.asarray(x, dtype=np.float32)
    W1 = np.asarray(W1, dtype=np.float32)
    b1 = np.asarray(b1, dtype=np.float32)
    W2 = np.asarray(W2, dtype=np.float32)
    b2 = np.asarray(b2, dtype=np.float32)
    W3 = np.asarray(W3, dtype=np.float32)
    b3 = np.asarray(b3, dtype=np.float32)
    p = _plan(src, dst)

    inv_perm = p["inv_perm"]
    xN = np.ascontiguousarray(x[inv_perm])                    # [N, DIN] new ids
    iota = np.tile(np.arange(128, dtype=np.float32), (128, 1))
    w1s = _rearrange_w(W1, 2)
    w2s = _rearrange_w(W2, 4)
    W3top = np.zeros((HID, M3P), np.float32)
    W3bot = np.zeros((HID, M3P), np.float32)
    W3top[:, :DOUT] = W3[:HID]
    W3bot[:, :DOUT] = W3[HID:]
    w3ts = _rearrange_w(W3top, 2)
    w3bs = _rearrange_w(W3bot, 2)
    b12 = np.stack([b1[:128], b1[128:], b2[:128], b2[128:]], axis=1).astype(np.float32)
    b3b = np.zeros((128, M3P), np.float32)
    b3b[:, :DOUT] = b3[None, :DOUT]

    in_maps = []
    for c in range(NCORES):
        xT_own = np.ascontiguousarray(xN[c * NPC:(c + 1) * NPC].T)
        in_maps.append({
            "x_nm": xN, "xT_own": xT_own,
            "idx": p["idx_pack"][c], "dstloc": p["dstloc_pack"][c],
            "invdst": p["invdst_pack"][c], "iota": iota,
            "w1": w1s, "w2": w2s, "w3t": w3ts, "w3b": w3bs,
            "b12": b12, "b3b": b3b,
        })
    return p, in_maps


def kernel(x, W1, b1, W2, b2, W3, b3, src, dst):
    p, in_maps = _prepare(x, W1, b1, W2, b2, W3, b3, src, dst)
    nc = _build(p["T_A"], p["T_B"])
    import os
    trace = bool(os.environ.get("KERNEL_TRACE"))
    tdir = os.environ.get("KERNEL_TRACE_DIR") or None
    if tdir:
        os.makedirs(tdir, exist_ok=True)
    res = run_bass_kernel_spmd(nc, in_maps, core_ids=list(range(NCORES)),
                               trace=trace, tmpdir=tdir)
    global LAST_EXEC_NS, LAST_RESULT
    LAST_EXEC_NS = res.exec_time_ns
    LAST_RESULT = res

    out_new = np.concatenate([res.results[c]["out"] for c in range(NCORES)], axis=0)
    return out_new[p["perm"]].astype(np.float32)



# revision 7
# speedup vs baseline: 2.9690x; 2.9690x over previous
"""Distributed 3-layer GraphSAGE (mean aggregator) on 8 TRN2 NeuronCores.

Strategy (graph/data parallel, per spec sharding hint):
  - Host: relabel nodes into 8 cores x 40 windows of 125 nodes with balanced
    in-degree; sort edges by (core, window, src-class); pad each (window,
    class) run to uniform tile counts -> fully static SPMD program.
  - Device, per layer: dma_gather edge source rows (bf16) from a replicated
    node-major feature table in HBM across 4 SWDGE queues; one-hot selection
    matrices built in bf16 on DVE (8 tiles per batched tensor_tensor via a
    stride-0 broadcast); segment-sum on the TensorEngine into PSUM per
    window; inv-degree applied on the psum->SBUF copy (Act engine);
    transform matmuls in bf16; AllGather (Shared-output) rebuilds the
    replicated table between layers.
  - Layer 3 uses transform-before-aggregate (m3 = h2 @ W3_bot, 47->128 pad)
    and adds the self term into a second PSUM, combined exactly with
    scalar_tensor_tensor.
"""
import numpy as np

import concourse.bacc as bacc
import concourse.mybir as mybir
import concourse.tile as tile
from concourse import bass
from concourse.bass_utils import run_bass_kernel_spmd
from concourse.library_config import mlp
from concourse.masks import make_identity

# ---- problem constants (hardcoded per contest rules) ----
N = 40000
E = 640000
DIN, HID, DOUT = 128, 256, 47
M3P = 128         # padded width of layer-3 edge features
NCORES = 8
WN = 125          # nodes per window (<= 128 PSUM partitions)
NW = 40           # windows per core
NPC = WN * NW     # 5000 nodes per core
SPLIT = 20000     # edge class split on src new-id (core-boundary aligned)
PAD_LOC = 126     # dead psum row for padding edges
CT = 8            # gather chunk size (tiles of 128 edges); 1024 idx/call
                  # is the SWDGE descriptor-ring capacity limit per dma_gather

F32 = mybir.dt.float32
BF16 = mybir.dt.bfloat16
I16 = mybir.dt.int16
AF = mybir.ActivationFunctionType
ALU = mybir.AluOpType

LAST_EXEC_NS = None
LAST_RESULT = None


# ======================= host-side planning =======================

def _plan(src, dst):
    import heapq
    src = np.asarray(src, dtype=np.int64)
    dst = np.asarray(dst, dtype=np.int64)
    deg = np.bincount(dst, minlength=N).astype(np.int64)

    nbins = NCORES * NW
    order = np.argsort(-deg, kind="stable")
    heap = [(0, b) for b in range(nbins)]
    heapq.heapify(heap)
    counts = np.zeros(nbins, dtype=np.int64)
    bin_of = np.empty(N, dtype=np.int64)
    spill = []
    for n in order:
        while True:
            load, b = heapq.heappop(heap)
            if counts[b] < WN:
                break
            spill.append((load, b))
        bin_of[n] = b
        counts[b] += 1
        if counts[b] < WN:
            heapq.heappush(heap, (load + int(deg[n]), b))
        for item in spill:
            heapq.heappush(heap, item)
        spill.clear()

    slot_in_bin = np.zeros(nbins, dtype=np.int64)
    perm = np.empty(N, dtype=np.int64)  # old -> new
    for n in range(N):
        b = bin_of[n]
        perm[n] = (b // NW) * NPC + (b % NW) * WN + slot_in_bin[b]
        slot_in_bin[b] += 1
    inv_perm = np.empty(N, dtype=np.int64)
    inv_perm[perm] = np.arange(N)

    srcN = perm[src]
    dstN = perm[dst]
    invdeg = np.zeros(N, dtype=np.float32)
    nz = deg > 0
    invdeg[nz] = (1.0 / deg[nz]).astype(np.float32)
    invdegN = invdeg[inv_perm]

    core_e = dstN // NPC
    win_e = (dstN % NPC) // WN
    loc_e = dstN % WN
    cls_e = (srcN >= SPLIT).astype(np.int64)
    key = (core_e * NW + win_e) * 2 + cls_e
    order_e = np.argsort(key, kind="stable")
    key_s = key[order_e]
    srcN_s = srcN[order_e]
    loc_s = loc_e[order_e]
    cnt = np.bincount(key_s, minlength=nbins * 2)
    starts = np.zeros(nbins * 2 + 1, dtype=np.int64)
    np.cumsum(cnt, out=starts[1:])

    T_A = int(np.ceil(cnt[0::2].max() / 128))
    T_B = int(np.ceil(cnt[1::2].max() / 128))
    LA, LB = NW * T_A * 128, NW * T_B * 128
    L = LA + LB
    NT = L // 128

    idx16 = np.zeros((NCORES, L), dtype=np.int16)
    dstloc = np.full((NCORES, L), PAD_LOC, dtype=np.float32)
    for c in range(NCORES):
        for w in range(NW):
            for s, (T, base_off) in enumerate(((T_A, 0), (T_B, LA))):
                k = (c * NW + w) * 2 + s
                e0, e1 = starts[k], starts[k + 1]
                n = e1 - e0
                off = base_off + w * T * 128
                sv = srcN_s[e0:e1]
                idx16[c, off:off + n] = (sv - (SPLIT if s else 0)).astype(np.int16)
                dstloc[c, off:off + n] = loc_s[e0:e1].astype(np.float32)

    idx_pack = np.empty((NCORES, 128, L // 16), dtype=np.int16)
    dstloc_pack = np.empty((NCORES, 128, NT), dtype=np.float32)
    for c in range(NCORES):
        blk = idx16[c].reshape(L // 16, 16).T
        idx_pack[c] = np.tile(blk, (8, 1))
        dstloc_pack[c] = dstloc[c].reshape(NT, 128).T

    # inverse in-degree per (window slot, window) for each core
    invwin = np.zeros((NCORES, 128, NW), dtype=np.float32)
    for c in range(NCORES):
        blkw = invdegN[c * NPC:(c + 1) * NPC].reshape(NW, WN).T  # [WN, NW]
        invwin[c, :WN, :] = blkw

    return dict(
        perm=perm, inv_perm=inv_perm, T_A=T_A, T_B=T_B,
        idx_pack=idx_pack, dstloc_pack=dstloc_pack, invwin=invwin,
    )


def _rearrange_w(W, kchunks):
    """[K, M] -> [128, kchunks*M] with k-chunk blocks along free dim."""
    K, M = W.shape
    assert K == kchunks * 128
    return np.ascontiguousarray(
        W.reshape(kchunks, 128, M).transpose(1, 0, 2).reshape(128, kchunks * M)
    ).astype(np.float32)


def _bf16(a):
    import ml_dtypes
    return np.asarray(a, dtype=np.float32).astype(ml_dtypes.bfloat16)


# ======================= device program =======================

def _build(T_A, T_B):
    import os
    MAXW = int(os.environ.get("KERNEL_MAXW", NW))
    NLAYERS = int(os.environ.get("KERNEL_NLAYERS", 3))
    SHARED_AG = os.environ.get("KERNEL_SHARED_AG", "1") == "1"
    nc = bacc.Bacc("TRN2", num_devices=NCORES, num_swdge_queues=4)
    NT_A, NT_B = NW * T_A, NW * T_B
    NT = NT_A + NT_B
    L = NT * 128
    assert NT_A % CT == 0 and NT_B % CT == 0

    # ---- kernel I/O ----
    x_nm = nc.dram_tensor("x_nm", [N, DIN], BF16, kind="ExternalInput")
    xT_own_d = nc.dram_tensor("xT_own", [128, NPC], BF16, kind="ExternalInput")
    idx_d = nc.dram_tensor("idx", [128, L // 16], I16, kind="ExternalInput")
    dstloc_d = nc.dram_tensor("dstloc", [128, NT], F32, kind="ExternalInput")
    invwin_d = nc.dram_tensor("invwin", [128, NW], F32, kind="ExternalInput")
    iota_d = nc.dram_tensor("iota", [128, CT * 128], F32, kind="ExternalInput")
    w1_d = nc.dram_tensor("w1", [128, 2 * HID], BF16, kind="ExternalInput")
    w2_d = nc.dram_tensor("w2", [128, 4 * HID], BF16, kind="ExternalInput")
    w3t_d = nc.dram_tensor("w3t", [128, 2 * M3P], BF16, kind="ExternalInput")
    w3b_d = nc.dram_tensor("w3b", [128, 2 * M3P], BF16, kind="ExternalInput")
    b12_d = nc.dram_tensor("b12", [128, 4], F32, kind="ExternalInput")
    b3b_d = nc.dram_tensor("b3b", [128, M3P], F32, kind="ExternalInput")
    out_d = nc.dram_tensor("out", [NPC, DOUT], F32, kind="ExternalOutput")

    QMAP = ((0, 2), (1, 3))   # class -> swdge queues (alternating per chunk)

    with tile.TileContext(nc) as tc:
        with (
            tc.tile_pool(name="persist", bufs=1) as PP,
            tc.tile_pool(name="dram", bufs=1, space="DRAM") as DP,
            tc.tile_pool(name="psA", bufs=2, space="PSUM") as PSA,
            tc.tile_pool(name="psT", bufs=2, space="PSUM") as PST,
            tc.tile_pool(name="ebufA", bufs=4) as PEA,
            tc.tile_pool(name="ebufB", bufs=4) as PEB,
            tc.tile_pool(name="spA", bufs=4) as PSPA,
            tc.tile_pool(name="spB", bufs=4) as PSPB,
            tc.tile_pool(name="tmp", bufs=2) as PT,
        ):
            nc.gpsimd.load_library(mlp)

            # persistent SBUF
            idx_sb = PP.tile([128, L // 16], I16)
            dstloc_sb = PP.tile([128, NT], F32)
            invwin_sb = PP.tile([128, NW], F32)
            iota_sb = PP.tile([128, CT * 128], F32)
            xT_own = PP.tile([128, NPC], BF16)
            w1_sb = PP.tile([128, 2 * HID], BF16)
            w2_sb = PP.tile([128, 4 * HID], BF16)
            w3t_sb = PP.tile([128, 2 * M3P], BF16)
            w3b_sb = PP.tile([128, 2 * M3P], BF16)
            b12_sb = PP.tile([128, 4], F32)
            b3b_sb = PP.tile([128, M3P], F32)
            ident = PP.tile([128, 128], BF16)
            h1T = [PP.tile([128, NPC], BF16, name=f"h1T{c}", tag=f"h1T{c}")
                   for c in range(2)]
            h2T = [PP.tile([128, NPC], BF16, name=f"h2T{c}", tag=f"h2T{c}")
                   for c in range(2)]

            for sb, dr in ((idx_sb, idx_d), (dstloc_sb, dstloc_d),
                           (invwin_sb, invwin_d), (iota_sb, iota_d),
                           (xT_own, xT_own_d),
                           (w1_sb, w1_d), (w2_sb, w2_d), (w3t_sb, w3t_d),
                           (w3b_sb, w3b_d), (b12_sb, b12_d), (b3b_sb, b3b_d)):
                nc.sync.dma_start(sb[:], dr[:])
            make_identity(nc, ident[:])

            # DRAM intermediates
            ag_space = "Shared" if SHARED_AG else "Local"
            h1_own = DP.tile([NPC, HID], BF16)
            h1_full = DP.tile([N, HID], BF16, addr_space=ag_space)
            m3_own = DP.tile([NPC, M3P], BF16)
            m3_full = DP.tile([N, M3P], BF16, addr_space=ag_space)

            # ---------- generic aggregation pass ----------
            def agg_layer(tableA, tableB, d, epilogue):
                """For each window: psum[seg, d] = sum_e S[e,seg]^T E[e, d]
                (one-hot S in bf16, inv-degree applied in the epilogue)."""
                issued = [0, 0]   # chunks issued per class
                bufs = [{}, {}]   # chunk idx -> (ebuf, sp)
                streams = (
                    (0, T_A, 0, NT_A, tableA, PEA, PSPA),
                    (1, T_B, NT_A, NT_B, tableB, PEB, PSPB),
                )

                def ensure_chunk(s, tix):
                    _, T, tile_off, nt, table, pool, sppool = streams[s]
                    c = tix // CT
                    while issued[s] <= c:
                        cc = issued[s]
                        t0 = cc * CT
                        ebuf = pool.tile([128, CT * d], BF16, tag=f"eb{s}")
                        col0 = (tile_off + t0) * 8  # 128 idx / 16 per col
                        nidx = CT * 128
                        nc.gpsimd.dma_gather(
                            ebuf[:].rearrange("p (t e) -> p t e", e=d),
                            table,
                            idx_sb[:, col0:col0 + nidx // 16],
                            nidx, nidx, d,
                            queue_num=QMAP[s][cc % 2],
                        )
                        sp = sppool.tile([128, CT * 128], BF16, tag=f"sp{s}")
                        a0 = iota_sb[:].rearrange("p (t c) -> p t c", c=128)
                        a1 = dstloc_sb[:, tile_off + t0:tile_off + t0 + CT] \
                            .rearrange("p (t o) -> p t o", o=1)
                        a0b, a1b = bass.broadcast_tensor_aps(a0, a1)
                        nc.vector.tensor_tensor(
                            sp[:].rearrange("p (t c) -> p t c", c=128),
                            a0b, a1b, op=ALU.is_equal)
                        bufs[s][cc] = (ebuf, sp)
                        issued[s] += 1
                    return bufs[s][c]

                for w in range(min(NW, MAXW)):
                    psum = PSA.tile([128, d], F32, tag="agg")
                    n_ent = T_A + T_B
                    i = 0
                    for s, T, tile_off, nt, table, pool, sppool in streams:
                        for j in range(T):
                            tix = w * T + j
                            ebuf, sp = ensure_chunk(s, tix)
                            slot = tix % CT
                            nc.tensor.matmul(
                                psum[:], lhsT=sp[:, slot * 128:(slot + 1) * 128],
                                rhs=ebuf[:, slot * d:(slot + 1) * d],
                                start=(i == 0),
                                stop=(i == n_ent - 1),
                            )
                            i += 1
                    epilogue(w, psum)

            # ---------- layer 1 ----------
            def epi1(w, psum):
                ws = slice(w * WN, (w + 1) * WN)
                # mean (inv-degree fold) then feature-major transpose
                meanw = PT.tile([128, DIN], BF16, tag="mean1")
                nc.scalar.activation(meanw[:], psum[:], AF.Copy,
                                     scale=invwin_sb[:, w:w + 1])
                pt = PST.tile([128, 128], BF16, tag="trb")
                nc.tensor.transpose(pt[:], meanw[:], ident[:])
                meanT = PT.tile([128, 128], BF16, tag="meanT1")
                nc.scalar.copy(meanT[:], pt[:])
                h1nm = PT.tile([128, HID], BF16, tag="h1nm")
                for dc in range(2):
                    ptr = PST.tile([128, WN], F32, tag="tr2")
                    nc.tensor.matmul(ptr[:], lhsT=w1_sb[:, dc * 128:dc * 128 + 128],
                                     rhs=xT_own[:, ws], start=True, stop=False)
                    nc.tensor.matmul(ptr[:], lhsT=w1_sb[:, HID + dc * 128:HID + dc * 128 + 128],
                                     rhs=meanT[:, :WN], start=False, stop=True)
                    nc.scalar.activation(h1T[dc][:, ws], ptr[:], AF.Relu,
                                         bias=b12_sb[:, dc:dc + 1])
                    pt2 = PST.tile([128, 128], BF16, tag="trb")
                    nc.tensor.transpose(pt2[:WN, :], h1T[dc][:, ws], ident[:])
                    nc.scalar.copy(h1nm[:WN, dc * 128:dc * 128 + 128], pt2[:WN, :])
                nc.sync.dma_start(h1_own[w * WN:(w + 1) * WN, :], h1nm[:WN, :])

            agg_layer(x_nm[:], x_nm[SPLIT:, :], DIN, epi1)
            if NLAYERS >= 2:
                nc.gpsimd.collective_compute(
                    "AllGather", ALU.bypass,
                    replica_groups=[list(range(NCORES))],
                    ins=[h1_own.opt()], outs=[h1_full.opt()],
                )

            # ---------- layer 2 (+ m3 transform) ----------
            def epi2(w, psum):
                ws = slice(w * WN, (w + 1) * WN)
                meanw = PT.tile([128, HID], BF16, tag="mean2")
                nc.scalar.activation(meanw[:], psum[:], AF.Copy,
                                     scale=invwin_sb[:, w:w + 1])
                meanT = PT.tile([128, 2 * 128], BF16, tag="meanT2")
                for dc in range(2):
                    pt = PST.tile([128, 128], BF16, tag="trb")
                    nc.tensor.transpose(pt[:], meanw[:, dc * 128:(dc + 1) * 128], ident[:])
                    nc.scalar.copy(meanT[:, dc * 128:(dc + 1) * 128], pt[:])
                for dc in range(2):
                    ptr = PST.tile([128, WN], F32, tag="tr2")
                    for k in range(2):   # h1T chunks
                        nc.tensor.matmul(
                            ptr[:], lhsT=w2_sb[:, k * HID + dc * 128:k * HID + dc * 128 + 128],
                            rhs=h1T[k][:, ws], start=(k == 0), stop=False)
                    for k in range(2):   # meanT chunks
                        nc.tensor.matmul(
                            ptr[:], lhsT=w2_sb[:, (2 + k) * HID + dc * 128:(2 + k) * HID + dc * 128 + 128],
                            rhs=meanT[:, k * 128:k * 128 + WN], start=False, stop=(k == 1))
                    nc.scalar.activation(h2T[dc][:, ws], ptr[:], AF.Relu,
                                         bias=b12_sb[:, 2 + dc:3 + dc])
                # m3 = h2 @ W3_bot  (feature-major then node-major)
                pm = PST.tile([128, WN], F32, tag="tr2")
                for k in range(2):
                    nc.tensor.matmul(pm[:M3P, :], lhsT=w3b_sb[:, k * M3P:(k + 1) * M3P],
                                     rhs=h2T[k][:, ws], start=(k == 0), stop=(k == 1))
                m3T_w = PT.tile([128, WN], BF16, tag="m3T")
                nc.scalar.copy(m3T_w[:M3P, :], pm[:M3P, :])
                pt3 = PST.tile([128, 128], BF16, tag="trb")
                nc.tensor.transpose(pt3[:WN, :M3P], m3T_w[:M3P, :], ident[:M3P, :M3P])
                m3nm = PT.tile([128, M3P], BF16, tag="m3nm")
                nc.scalar.copy(m3nm[:WN, :], pt3[:WN, :M3P])
                nc.sync.dma_start(m3_own[w * WN:(w + 1) * WN, :], m3nm[:WN, :])

            if NLAYERS >= 2:
                agg_layer(h1_full[:], h1_full[SPLIT:, :], HID, epi2)
            if NLAYERS >= 3:
                nc.gpsimd.collective_compute(
                    "AllGather", ALU.bypass,
                    replica_groups=[list(range(NCORES))],
                    ins=[m3_own.opt()], outs=[m3_full.opt()],
                )

            # ---------- layer 3 ----------
            def epi3(w, psum):
                # psum holds sum(m3[src]) [seg, M3P]; compute the self term
                # h2 @ W3_top into a second psum, combine exactly:
                # out = psum * invdeg + self, then add bias.
                ws = slice(w * WN, (w + 1) * WN)
                pself = PST.tile([128, M3P], F32, tag="tr")
                for k in range(2):
                    nc.tensor.matmul(pself[:WN, :], lhsT=h2T[k][:, ws],
                                     rhs=w3t_sb[:, k * M3P:(k + 1) * M3P],
                                     start=(k == 0), stop=(k == 1))
                selfb = PT.tile([128, DOUT], F32, tag="selfb")
                nc.vector.tensor_tensor(selfb[:WN, :], pself[:WN, :DOUT],
                                        b3b_sb[:WN, :DOUT], op=ALU.add)
                out_w = PT.tile([128, DOUT], F32, tag="outw")
                nc.vector.scalar_tensor_tensor(
                    out_w[:WN, :], in0=psum[:WN, :DOUT],
                    scalar=invwin_sb[:WN, w:w + 1],
                    in1=selfb[:WN, :],
                    op0=ALU.mult, op1=ALU.add)
                nc.sync.dma_start(out_d[w * WN:(w + 1) * WN, :], out_w[:WN, :])

            if NLAYERS >= 3:
                agg_layer(m3_full[:], m3_full[SPLIT:, :], M3P, epi3)

    nc.compile()
    return nc


# ======================= top-level entry =======================

def _prepare(x, W1, b1, W2, b2, W3, b3, src, dst):
    x = np.asarray(x, dtype=np.float32)
    W1 = np.asarray(W1, dtype=np.float32)
    b1 = np.asarray(b1, dtype=np.float32)
    W2 = np.asarray(W2, dtype=np.float32)
    b2 = np.asarray(b2, dtype=np.float32)
    W3 = np.asarray(W3, dtype=np.float32)
    b3 = np.asarray(b3, dtype=np.float32)
    p = _plan(src, dst)

    inv_perm = p["inv_perm"]
    xN = _bf16(x[inv_perm])                                   # [N, DIN] new ids
    iota = np.tile(np.arange(128, dtype=np.float32), (128, CT))
    w1s = _bf16(_rearrange_w(W1, 2))
    w2s = _bf16(_rearrange_w(W2, 4))
    W3top = np.zeros((HID, M3P), np.float32)
    W3bot = np.zeros((HID, M3P), np.float32)
    W3top[:, :DOUT] = W3[:HID]
    W3bot[:, :DOUT] = W3[HID:]
    w3ts = _bf16(_rearrange_w(W3top, 2))
    w3bs = _bf16(_rearrange_w(W3bot, 2))
    b12 = np.stack([b1[:128], b1[128:], b2[:128], b2[128:]], axis=1).astype(np.float32)
    b3b = np.zeros((128, M3P), np.float32)
    b3b[:, :DOUT] = b3[None, :DOUT]

    in_maps = []
    for c in range(NCORES):
        xT_own = np.ascontiguousarray(xN[c * NPC:(c + 1) * NPC].T)
        in_maps.append({
            "x_nm": xN, "xT_own": xT_own,
            "idx": p["idx_pack"][c], "dstloc": p["dstloc_pack"][c],
            "invwin": p["invwin"][c], "iota": iota,
            "w1": w1s, "w2": w2s, "w3t": w3ts, "w3b": w3bs,
            "b12": b12, "b3b": b3b,
        })
    return p, in_maps


def kernel(x, W1, b1, W2, b2, W3, b3, src, dst):
    p, in_maps = _prepare(x, W1, b1, W2, b2, W3, b3, src, dst)
    nc = _build(p["T_A"], p["T_B"])
    import os
    trace = bool(os.environ.get("KERNEL_TRACE"))
    tdir = os.environ.get("KERNEL_TRACE_DIR") or None
    if tdir:
        os.makedirs(tdir, exist_ok=True)
    res = run_bass_kernel_spmd(nc, in_maps, core_ids=list(range(NCORES)),
                               trace=trace, tmpdir=tdir)
    global LAST_EXEC_NS, LAST_RESULT
    LAST_EXEC_NS = res.exec_time_ns
    LAST_RESULT = res

    out_new = np.concatenate([res.results[c]["out"] for c in range(NCORES)], axis=0)
    return out_new[p["perm"]].astype(np.float32)


# revision 10
# speedup vs baseline: 4.0651x; 1.3692x over previous
"""Distributed 3-layer GraphSAGE (mean aggregator) on 8 TRN2 NeuronCores.

Strategy (graph/data parallel, per spec sharding hint):
  - Host: relabel nodes into 8 cores x 40 windows of 125 nodes with balanced
    in-degree; sort edges by (core, window, src-class); pad each (window,
    class) run to uniform tile counts -> fully static SPMD program.
  - Device, per layer: dma_gather edge source rows (bf16) from a replicated
    node-major feature table in HBM across 4 SWDGE queues; one-hot selection
    matrices built in bf16 on DVE (8 tiles per batched tensor_tensor via a
    stride-0 broadcast); segment-sum on the TensorEngine into PSUM per
    window; inv-degree applied on the psum->SBUF copy (Act engine);
    transform matmuls in bf16; AllGather (Shared-output) rebuilds the
    replicated table between layers.
  - Layer 3 uses transform-before-aggregate (m3 = h2 @ W3_bot, 47->128 pad)
    and adds the self term into a second PSUM, combined exactly with
    scalar_tensor_tensor.
"""
import numpy as np

import concourse.bacc as bacc
import concourse.mybir as mybir
import concourse.tile as tile
from concourse import bass
from concourse.bass_utils import run_bass_kernel_spmd
from concourse.library_config import mlp
from concourse.masks import make_identity

# ---- problem constants (hardcoded per contest rules) ----
N = 40000
E = 640000
DIN, HID, DOUT = 128, 256, 47
M3P = 128         # padded width of layer-3 edge features
NCORES = 8
WN = 125          # nodes per window (<= 128 PSUM partitions)
NW = 40           # windows per core
NPC = WN * NW     # 5000 nodes per core
B_BASE = 10000    # class-B gather table base (idx int16: B covers [10000,40000))
A_MAX = 32768     # class-A idx limit (covers [0,32768)); overlap = flexible
PAD_LOC = 126     # dead psum row for padding edges
CT = 8            # gather chunk size (tiles of 128 edges); 1024 idx/call
                  # is the SWDGE descriptor-ring capacity limit per dma_gather

F32 = mybir.dt.float32
BF16 = mybir.dt.bfloat16
I16 = mybir.dt.int16
AF = mybir.ActivationFunctionType
ALU = mybir.AluOpType

LAST_EXEC_NS = None
LAST_RESULT = None


# ======================= host-side planning =======================

def _plan(src, dst):
    import heapq
    src = np.asarray(src, dtype=np.int64)
    dst = np.asarray(dst, dtype=np.int64)
    deg = np.bincount(dst, minlength=N).astype(np.int64)

    nbins = NCORES * NW
    order = np.argsort(-deg, kind="stable")
    heap = [(0, b) for b in range(nbins)]
    heapq.heapify(heap)
    counts = np.zeros(nbins, dtype=np.int64)
    bin_of = np.empty(N, dtype=np.int64)
    spill = []
    for n in order:
        while True:
            load, b = heapq.heappop(heap)
            if counts[b] < WN:
                break
            spill.append((load, b))
        bin_of[n] = b
        counts[b] += 1
        if counts[b] < WN:
            heapq.heappush(heap, (load + int(deg[n]), b))
        for item in spill:
            heapq.heappush(heap, item)
        spill.clear()

    slot_in_bin = np.zeros(nbins, dtype=np.int64)
    perm = np.empty(N, dtype=np.int64)  # old -> new
    for n in range(N):
        b = bin_of[n]
        perm[n] = (b // NW) * NPC + (b % NW) * WN + slot_in_bin[b]
        slot_in_bin[b] += 1
    inv_perm = np.empty(N, dtype=np.int64)
    inv_perm[perm] = np.arange(N)

    srcN = perm[src]
    dstN = perm[dst]
    invdeg = np.zeros(N, dtype=np.float32)
    nz = deg > 0
    invdeg[nz] = (1.0 / deg[nz]).astype(np.float32)
    invdegN = invdeg[inv_perm]

    core_e = dstN // NPC
    win_e = (dstN % NPC) // WN
    loc_e = dstN % WN
    bin_e = core_e * NW + win_e
    # int16 gather classes: A reads table[:A_MAX], B reads table[B_BASE:].
    # Edges with src in [B_BASE, A_MAX) are flexible; split them per bin so
    # cntA ~= cntB ~= total/2 (minimizes padded tile count).
    fixedA = srcN < B_BASE
    fixedB = srcN >= A_MAX
    flex = ~fixedA & ~fixedB
    nA = np.bincount(bin_e[fixedA], minlength=nbins)
    ntot = np.bincount(bin_e, minlength=nbins)
    nflex = np.bincount(bin_e[flex], minlength=nbins)
    aA = np.clip(ntot // 2 - nA, 0, nflex)   # flex edges sent to class A
    cls_e = np.where(fixedB, 1, 0).astype(np.int64)
    fidx = np.flatnonzero(flex)
    forder = fidx[np.argsort(bin_e[fidx], kind="stable")]
    fb = bin_e[forder]
    start_of = np.zeros(nbins, dtype=np.int64)
    np.cumsum(np.bincount(fb, minlength=nbins)[:-1], out=start_of[1:])
    rank = np.arange(len(forder)) - start_of[fb]
    cls_e[forder] = (rank >= aA[fb]).astype(np.int64)
    key = bin_e * 2 + cls_e
    order_e = np.argsort(key, kind="stable")
    key_s = key[order_e]
    srcN_s = srcN[order_e]
    loc_s = loc_e[order_e]
    cnt = np.bincount(key_s, minlength=nbins * 2)
    starts = np.zeros(nbins * 2 + 1, dtype=np.int64)
    np.cumsum(cnt, out=starts[1:])

    T_A = int(np.ceil(cnt[0::2].max() / 128))
    T_B = int(np.ceil(cnt[1::2].max() / 128))
    LA, LB = NW * T_A * 128, NW * T_B * 128
    L = LA + LB
    NT = L // 128

    idx16 = np.zeros((NCORES, L), dtype=np.int16)
    dstloc = np.full((NCORES, L), PAD_LOC, dtype=np.float32)
    for c in range(NCORES):
        for w in range(NW):
            for s, (T, base_off) in enumerate(((T_A, 0), (T_B, LA))):
                k = (c * NW + w) * 2 + s
                e0, e1 = starts[k], starts[k + 1]
                n = e1 - e0
                off = base_off + w * T * 128
                sv = srcN_s[e0:e1]
                idx16[c, off:off + n] = (sv - (B_BASE if s else 0)).astype(np.int16)
                dstloc[c, off:off + n] = loc_s[e0:e1].astype(np.float32)

    idx_pack = np.empty((NCORES, 128, L // 16), dtype=np.int16)
    dstloc_pack = np.empty((NCORES, 128, NT), dtype=np.float32)
    for c in range(NCORES):
        blk = idx16[c].reshape(L // 16, 16).T
        idx_pack[c] = np.tile(blk, (8, 1))
        dstloc_pack[c] = dstloc[c].reshape(NT, 128).T

    # inverse in-degree per (window slot, window) for each core
    invwin = np.zeros((NCORES, 128, NW), dtype=np.float32)
    for c in range(NCORES):
        blkw = invdegN[c * NPC:(c + 1) * NPC].reshape(NW, WN).T  # [WN, NW]
        invwin[c, :WN, :] = blkw

    return dict(
        perm=perm, inv_perm=inv_perm, T_A=T_A, T_B=T_B,
        idx_pack=idx_pack, dstloc_pack=dstloc_pack, invwin=invwin,
    )


def _rearrange_w(W, kchunks):
    """[K, M] -> [128, kchunks*M] with k-chunk blocks along free dim."""
    K, M = W.shape
    assert K == kchunks * 128
    return np.ascontiguousarray(
        W.reshape(kchunks, 128, M).transpose(1, 0, 2).reshape(128, kchunks * M)
    ).astype(np.float32)


def _bf16(a):
    import ml_dtypes
    return np.asarray(a, dtype=np.float32).astype(ml_dtypes.bfloat16)


# ======================= device program =======================

def _build(T_A, T_B):
    import os
    MAXW = int(os.environ.get("KERNEL_MAXW", NW))
    NLAYERS = int(os.environ.get("KERNEL_NLAYERS", 3))
    SHARED_AG = os.environ.get("KERNEL_SHARED_AG", "1") == "1"
    nc = bacc.Bacc("TRN2", num_devices=NCORES, num_swdge_queues=4)
    NT_A, NT_B = NW * T_A, NW * T_B
    NT = NT_A + NT_B
    L = NT * 128
    assert NT_A % CT == 0 and NT_B % CT == 0

    # ---- kernel I/O ----
    x_nm = nc.dram_tensor("x_nm", [N, DIN], BF16, kind="ExternalInput")
    xT_own_d = nc.dram_tensor("xT_own", [128, NPC], BF16, kind="ExternalInput")
    idx_d = nc.dram_tensor("idx", [128, L // 16], I16, kind="ExternalInput")
    dstloc_d = nc.dram_tensor("dstloc", [128, NT], BF16, kind="ExternalInput")
    invwin_d = nc.dram_tensor("invwin", [128, NW], F32, kind="ExternalInput")
    iota_d = nc.dram_tensor("iota", [128, CT * 128], BF16, kind="ExternalInput")
    w1_d = nc.dram_tensor("w1", [128, 2 * HID], BF16, kind="ExternalInput")
    w2_d = nc.dram_tensor("w2", [128, 4 * HID], BF16, kind="ExternalInput")
    w3t_d = nc.dram_tensor("w3t", [128, 2 * M3P], BF16, kind="ExternalInput")
    w3b_d = nc.dram_tensor("w3b", [128, 2 * M3P], BF16, kind="ExternalInput")
    b12_d = nc.dram_tensor("b12", [128, 4], F32, kind="ExternalInput")
    b3b_d = nc.dram_tensor("b3b", [128, M3P], F32, kind="ExternalInput")
    out_d = nc.dram_tensor("out", [NPC, DOUT], F32, kind="ExternalOutput")

    QMAP = ((0, 2), (1, 3))   # class -> swdge queues (alternating per chunk)

    with tile.TileContext(nc) as tc:
        with (
            tc.tile_pool(name="persist", bufs=1) as PP,
            tc.tile_pool(name="dram", bufs=1, space="DRAM") as DP,
            tc.tile_pool(name="psA", bufs=2, space="PSUM") as PSA,
            tc.tile_pool(name="psT", bufs=2, space="PSUM") as PST,
            tc.tile_pool(name="ebufA", bufs=4) as PEA,
            tc.tile_pool(name="ebufB", bufs=4) as PEB,
            tc.tile_pool(name="spA", bufs=4) as PSPA,
            tc.tile_pool(name="spB", bufs=4) as PSPB,
            tc.tile_pool(name="tmp", bufs=2) as PT,
        ):
            nc.gpsimd.load_library(mlp)

            # persistent SBUF
            idx_sb = PP.tile([128, L // 16], I16)
            dstloc_sb = PP.tile([128, NT], BF16)
            invwin_sb = PP.tile([128, NW], F32)
            iota_sb = PP.tile([128, CT * 128], BF16)
            xT_own = PP.tile([128, NPC], BF16)
            w1_sb = PP.tile([128, 2 * HID], BF16)
            w2_sb = PP.tile([128, 4 * HID], BF16)
            w3t_sb = PP.tile([128, 2 * M3P], BF16)
            w3b_sb = PP.tile([128, 2 * M3P], BF16)
            b12_sb = PP.tile([128, 4], F32)
            b3b_sb = PP.tile([128, M3P], F32)
            ident = PP.tile([128, 128], BF16)
            h1T = [PP.tile([128, NPC], BF16, name=f"h1T{c}", tag=f"h1T{c}")
                   for c in range(2)]
            h2T = [PP.tile([128, NPC], BF16, name=f"h2T{c}", tag=f"h2T{c}")
                   for c in range(2)]

            for sb, dr in ((idx_sb, idx_d), (dstloc_sb, dstloc_d),
                           (invwin_sb, invwin_d), (iota_sb, iota_d),
                           (xT_own, xT_own_d),
                           (w1_sb, w1_d), (w2_sb, w2_d), (w3t_sb, w3t_d),
                           (w3b_sb, w3b_d), (b12_sb, b12_d), (b3b_sb, b3b_d)):
                nc.sync.dma_start(sb[:], dr[:])
            make_identity(nc, ident[:])
            nidx_reg = nc.gpsimd.to_reg(CT * 128)

            # DRAM intermediates
            ag_space = "Shared" if SHARED_AG else "Local"
            h1_own = DP.tile([NPC, HID], BF16)
            h1_full = DP.tile([N, HID], BF16, addr_space=ag_space)
            m3_own = DP.tile([NPC, M3P], BF16)
            m3_full = DP.tile([N, M3P], BF16, addr_space=ag_space)

            # ---------- generic aggregation pass ----------
            def agg_layer(tableA, tableB, d, epilogue):
                """For each window: psum[seg, d] = sum_e S[e,seg]^T E[e, d]
                (one-hot S in bf16, inv-degree applied in the epilogue)."""
                issued = [0, 0]   # chunks issued per class
                bufs = [{}, {}]   # chunk idx -> (ebuf, sp)
                streams = (
                    (0, T_A, 0, NT_A, tableA, PEA, PSPA),
                    (1, T_B, NT_A, NT_B, tableB, PEB, PSPB),
                )

                def ensure_chunk(s, tix):
                    _, T, tile_off, nt, table, pool, sppool = streams[s]
                    c = tix // CT
                    while issued[s] <= c:
                        cc = issued[s]
                        t0 = cc * CT
                        ebuf = pool.tile([128, CT * d], BF16, tag=f"eb{s}")
                        col0 = (tile_off + t0) * 8  # 128 idx / 16 per col
                        nidx = CT * 128
                        nc.gpsimd.dma_gather(
                            ebuf[:].rearrange("p (t e) -> p t e", e=d),
                            table,
                            idx_sb[:, col0:col0 + nidx // 16],
                            nidx, nidx_reg, d,
                            queue_num=QMAP[s][cc % 2],
                        )
                        sp = sppool.tile([128, CT * 128], BF16, tag=f"sp{s}")
                        a0 = iota_sb[:].rearrange("p (t c) -> p t c", c=128)
                        a1 = dstloc_sb[:, tile_off + t0:tile_off + t0 + CT] \
                            .rearrange("p (t o) -> p t o", o=1)
                        a0b, a1b = bass.broadcast_tensor_aps(a0, a1)
                        nc.vector.tensor_tensor(
                            sp[:].rearrange("p (t c) -> p t c", c=128),
                            a0b, a1b, op=ALU.is_equal)
                        bufs[s][cc] = (ebuf, sp)
                        issued[s] += 1
                    return bufs[s][c]

                for w in range(min(NW, MAXW)):
                    psum = PSA.tile([128, d], F32, tag="agg")
                    n_ent = T_A + T_B
                    i = 0
                    for s, T, tile_off, nt, table, pool, sppool in streams:
                        for j in range(T):
                            tix = w * T + j
                            ebuf, sp = ensure_chunk(s, tix)
                            slot = tix % CT
                            nc.tensor.matmul(
                                psum[:], lhsT=sp[:, slot * 128:(slot + 1) * 128],
                                rhs=ebuf[:, slot * d:(slot + 1) * d],
                                start=(i == 0),
                                stop=(i == n_ent - 1),
                            )
                            i += 1
                    epilogue(w, psum)

            # ---------- layer 1 ----------
            def epi1(w, psum):
                ws = slice(w * WN, (w + 1) * WN)
                # mean (inv-degree fold) then feature-major transpose
                meanw = PT.tile([128, DIN], BF16, tag="mean1")
                nc.scalar.activation(meanw[:], psum[:], AF.Copy,
                                     scale=invwin_sb[:, w:w + 1])
                pt = PST.tile([128, 128], BF16, tag="trb")
                nc.tensor.transpose(pt[:], meanw[:], ident[:])
                meanT = PT.tile([128, 128], BF16, tag="meanT1")
                nc.scalar.copy(meanT[:], pt[:])
                h1nm = PT.tile([128, HID], BF16, tag="h1nm")
                for dc in range(2):
                    ptr = PST.tile([128, WN], F32, tag="tr2")
                    nc.tensor.matmul(ptr[:], lhsT=w1_sb[:, dc * 128:dc * 128 + 128],
                                     rhs=xT_own[:, ws], start=True, stop=False)
                    nc.tensor.matmul(ptr[:], lhsT=w1_sb[:, HID + dc * 128:HID + dc * 128 + 128],
                                     rhs=meanT[:, :WN], start=False, stop=True)
                    nc.scalar.activation(h1T[dc][:, ws], ptr[:], AF.Relu,
                                         bias=b12_sb[:, dc:dc + 1])
                    pt2 = PST.tile([128, 128], BF16, tag="trb")
                    nc.tensor.transpose(pt2[:WN, :], h1T[dc][:, ws], ident[:])
                    nc.scalar.copy(h1nm[:WN, dc * 128:dc * 128 + 128], pt2[:WN, :])
                nc.sync.dma_start(h1_own[w * WN:(w + 1) * WN, :], h1nm[:WN, :])

            agg_layer(x_nm[:], x_nm[B_BASE:, :], DIN, epi1)
            if NLAYERS >= 2:
                nc.gpsimd.collective_compute(
                    "AllGather", ALU.bypass,
                    replica_groups=[list(range(NCORES))],
                    ins=[h1_own.opt()], outs=[h1_full.opt()],
                )

            # ---------- layer 2 (+ m3 transform) ----------
            def epi2(w, psum):
                ws = slice(w * WN, (w + 1) * WN)
                meanw = PT.tile([128, HID], BF16, tag="mean2")
                nc.scalar.activation(meanw[:], psum[:], AF.Copy,
                                     scale=invwin_sb[:, w:w + 1])
                meanT = PT.tile([128, 2 * 128], BF16, tag="meanT2")
                for dc in range(2):
                    pt = PST.tile([128, 128], BF16, tag="trb")
                    nc.tensor.transpose(pt[:], meanw[:, dc * 128:(dc + 1) * 128], ident[:])
                    nc.scalar.copy(meanT[:, dc * 128:(dc + 1) * 128], pt[:])
                for dc in range(2):
                    ptr = PST.tile([128, WN], F32, tag="tr2")
                    for k in range(2):   # h1T chunks
                        nc.tensor.matmul(
                            ptr[:], lhsT=w2_sb[:, k * HID + dc * 128:k * HID + dc * 128 + 128],
                            rhs=h1T[k][:, ws], start=(k == 0), stop=False)
                    for k in range(2):   # meanT chunks
                        nc.tensor.matmul(
                            ptr[:], lhsT=w2_sb[:, (2 + k) * HID + dc * 128:(2 + k) * HID + dc * 128 + 128],
                            rhs=meanT[:, k * 128:k * 128 + WN], start=False, stop=(k == 1))
                    nc.scalar.activation(h2T[dc][:, ws], ptr[:], AF.Relu,
                                         bias=b12_sb[:, 2 + dc:3 + dc])
                # m3 = h2 @ W3_bot  (feature-major then node-major)
                pm = PST.tile([128, WN], F32, tag="tr2")
                for k in range(2):
                    nc.tensor.matmul(pm[:M3P, :], lhsT=w3b_sb[:, k * M3P:(k + 1) * M3P],
                                     rhs=h2T[k][:, ws], start=(k == 0), stop=(k == 1))
                m3T_w = PT.tile([128, WN], BF16, tag="m3T")
                nc.scalar.copy(m3T_w[:M3P, :], pm[:M3P, :])
                pt3 = PST.tile([128, 128], BF16, tag="trb")
                nc.tensor.transpose(pt3[:WN, :M3P], m3T_w[:M3P, :], ident[:M3P, :M3P])
                m3nm = PT.tile([128, M3P], BF16, tag="m3nm")
                nc.scalar.copy(m3nm[:WN, :], pt3[:WN, :M3P])
                nc.sync.dma_start(m3_own[w * WN:(w + 1) * WN, :], m3nm[:WN, :])

            if NLAYERS >= 2:
                agg_layer(h1_full[:], h1_full[B_BASE:, :], HID, epi2)
            if NLAYERS >= 3:
                nc.gpsimd.collective_compute(
                    "AllGather", ALU.bypass,
                    replica_groups=[list(range(NCORES))],
                    ins=[m3_own.opt()], outs=[m3_full.opt()],
                )

            # ---------- layer 3 ----------
            def epi3(w, psum):
                # psum holds sum(m3[src]) [seg, M3P]; compute the self term
                # h2 @ W3_top into a second psum, combine exactly:
                # out = psum * invdeg + self, then add bias.
                ws = slice(w * WN, (w + 1) * WN)
                pself = PST.tile([128, M3P], F32, tag="tr")
                for k in range(2):
                    nc.tensor.matmul(pself[:WN, :], lhsT=h2T[k][:, ws],
                                     rhs=w3t_sb[:, k * M3P:(k + 1) * M3P],
                                     start=(k == 0), stop=(k == 1))
                selfb = PT.tile([128, DOUT], F32, tag="selfb")
                nc.vector.tensor_tensor(selfb[:WN, :], pself[:WN, :DOUT],
                                        b3b_sb[:WN, :DOUT], op=ALU.add)
                out_w = PT.tile([128, DOUT], F32, tag="outw")
                nc.vector.scalar_tensor_tensor(
                    out_w[:WN, :], in0=psum[:WN, :DOUT],
                    scalar=invwin_sb[:WN, w:w + 1],
                    in1=selfb[:WN, :],
                    op0=ALU.mult, op1=ALU.add)
                nc.sync.dma_start(out_d[w * WN:(w + 1) * WN, :], out_w[:WN, :])

            if NLAYERS >= 3:
                agg_layer(m3_full[:], m3_full[B_BASE:, :], M3P, epi3)

    nc.compile()
    return nc


# ======================= top-level entry =======================

def _prepare(x, W1, b1, W2, b2, W3, b3, src, dst):
    x = np.asarray(x, dtype=np.float32)
    W1 = np.asarray(W1, dtype=np.float32)
    b1 = np.asarray(b1, dtype=np.float32)
    W2 = np.asarray(W2, dtype=np.float32)
    b2 = np.asarray(b2, dtype=np.float32)
    W3 = np.asarray(W3, dtype=np.float32)
    b3 = np.asarray(b3, dtype=np.float32)
    p = _plan(src, dst)

    inv_perm = p["inv_perm"]
    xN = _bf16(x[inv_perm])                                   # [N, DIN] new ids
    iota = _bf16(np.tile(np.arange(128, dtype=np.float32), (128, CT)))
    w1s = _bf16(_rearrange_w(W1, 2))
    w2s = _bf16(_rearrange_w(W2, 4))
    W3top = np.zeros((HID, M3P), np.float32)
    W3bot = np.zeros((HID, M3P), np.float32)
    W3top[:, :DOUT] = W3[:HID]
    W3bot[:, :DOUT] = W3[HID:]
    w3ts = _bf16(_rearrange_w(W3top, 2))
    w3bs = _bf16(_rearrange_w(W3bot, 2))
    b12 = np.stack([b1[:128], b1[128:], b2[:128], b2[128:]], axis=1).astype(np.float32)
    b3b = np.zeros((128, M3P), np.float32)
    b3b[:, :DOUT] = b3[None, :DOUT]

    in_maps = []
    for c in range(NCORES):
        xT_own = np.ascontiguousarray(xN[c * NPC:(c + 1) * NPC].T)
        in_maps.append({
            "x_nm": xN, "xT_own": xT_own,
            "idx": p["idx_pack"][c], "dstloc": _bf16(p["dstloc_pack"][c]),
            "invwin": p["invwin"][c], "iota": iota,
            "w1": w1s, "w2": w2s, "w3t": w3ts, "w3b": w3bs,
            "b12": b12, "b3b": b3b,
        })
    return p, in_maps


def kernel(x, W1, b1, W2, b2, W3, b3, src, dst):
    p, in_maps = _prepare(x, W1, b1, W2, b2, W3, b3, src, dst)
    nc = _build(p["T_A"], p["T_B"])
    import os
    trace = bool(os.environ.get("KERNEL_TRACE"))
    tdir = os.environ.get("KERNEL_TRACE_DIR") or None
    if tdir:
        os.makedirs(tdir, exist_ok=True)
    res = run_bass_kernel_spmd(nc, in_maps, core_ids=list(range(NCORES)),
                               trace=trace, tmpdir=tdir)
    global LAST_EXEC_NS, LAST_RESULT
    LAST_EXEC_NS = res.exec_time_ns
    LAST_RESULT = res

    out_new = np.concatenate([res.results[c]["out"] for c in range(NCORES)], axis=0)
    return out_new[p["perm"]].astype(np.float32)
